# revision 20
# baseline (speedup 1.0000x reference)
"""GaussianMask kernel for Trainium2 (Bass/Tile), SPMD over 8 NeuronCores.

Problem: X [4,3,512,512] f32 -> K [4,3,24,512,512] f32 where
  K[b,c,k,h,w] = exp(-0.5 * (Xpad[b,c,h+dy,w+dx] - X[b,c,h,w])^2)
for the 24 5x5 neighbor offsets (center excluded), zero padding of 2.

Strategy (pure data parallel, no communication):
  - 12 images (B*C) x 512 rows -> 24 half-images of 256 rows; 3 per core.
  - Per half-image tile: SBUF layout [128 partitions, 6 rows x 516 cols]
    where partition p holds padded rows 2p..2p+5 (its 2 output rows plus
    +-2 halo) of the 516-wide padded image. All 24 neighbor shifts are then
    pure free-dim offsets; no cross-partition movement is needed.
  - The kernel is a flat per-offset pipeline: sub (DVE, bf16 2x mode) ->
    in-place square (DVE) -> Exp(scale=-0.5) (ACT, f32 out) -> per-offset
    store DMA. The total HBM traffic is dominated by the 37.75 MB of f32
    output per core, so the goal is to keep the store stream dense from
    t~=8us on; per-offset stores (128 descs x 4KB) keep the DMA queue fed
    as soon as the first exp retires instead of batching 12 exps per store.
  - The odd-dx shifted slab (for 4B-aligned DVE 2x reads) is built on-chip
    with one DVE tensor_copy per tile (4x copy mode) instead of a second
    HBM load, halving input DMA traffic. The copy also doubles as the
    DVE-side observer of the tile's load semaphore, so the subs that follow
    need only their single d-slot-recycle ACT wait (walrus can encode only
    ONE sem wait on DVE/DMA instructions).
"""

import numpy as np

import concourse.bass as bass
import concourse.mybir as mybir
import concourse.tile as tile
from concourse.bass_utils import run_bass_kernel_spmd

N_CORES = 8
B, C, H, W = 4, 3, 512, 512
PAD = 2
PW = W + 2 * PAD          # 516 padded width
HALF = 256                # rows per half-image tile
TILES = 3                 # half-images per core
SLAB_ROWS = 6             # 2 output rows + 4 halo rows per partition
SLAB = SLAB_ROWS * PW     # 3096 elements per partition per tile
IN_TILE = 2 * PW * 127 + SLAB   # flat elems per half-image input (131064+3096)
IN_LEN = TILES * IN_TILE
OUT_TILE = HALF * W       # 131072 f32 per (tile, offset) output block

D_BUFS = 12               # rotating diff buffers (bf16, 2KB/partition each)
O_BUFS = 10               # rotating output buffers (f32, 4KB/partition each)

# 24 neighbor offsets, center (2,2) excluded, in reference order.
OFFSETS = []
for k in range(24):
    idx = k if k < 12 else k + 1
    OFFSETS.append((idx // 5, idx % 5))

_CACHED = None


def _patch_tail_drain():
    """Split the kernel-tail drain's sem waits across one drain per sem.

    Tile attaches every outstanding semaphore wait to a single Drain
    instruction, but walrus' CTRL codegen can only encode a bounded number
    of sync waits per instruction and dies with "Too many sync wait
    commands". One drain per nonzero proc keeps every instruction at a
    single wait.
    """
    from concourse.tile import TileContext
    from concourse.vector_clock import ScopedClock, VectorClock

    if getattr(TileContext, "_tail_drain_patched", False):
        return

    def _drain_and_barrier(self, tick_clock, wait_clock):
        gc = tick_clock.global_clock
        vals = eval(repr(gc).replace("VectorClock", ""))
        for i, v in enumerate(vals):
            if v <= 0:
                continue
            sub = [0] * len(vals)
            sub[i] = v
            drain_inst = self.nc.sync.drain()
            wait_clock.add_sem_waits(
                drain_inst.ins, ScopedClock({None: VectorClock(sub)}))
        self.nc.all_engine_barrier()
        assert self.sems is not None
        popped = self.nc._tile_sem_poison_stack.pop()
        assert popped is self._sem_poison
        self.nc.clear_and_free_semaphores(list(self.sems.allocated().values()))
        self.nc.all_engine_barrier()

    TileContext._drain_and_barrier = _drain_and_barrier
    TileContext._tail_drain_patched = True


def _build_bass():
    _patch_tail_drain()
    nc = bass.Bass("TRN2", target_bir_lowering=False, debug=False,
                   num_devices=N_CORES, dynamic_dma_scratch_size=4096)
    x_h = nc.dram_tensor("x", [IN_LEN], mybir.dt.bfloat16,
                         kind="ExternalInput")
    y_h = nc.dram_tensor("y", [TILES * 24 * OUT_TILE], mybir.dt.float32,
                         kind="ExternalOutput")

    f32 = mybir.dt.float32
    bf16 = mybir.dt.bfloat16

    # (nop, store) pairs: after scheduling, each store's ACT data wait is
    # stripped — the SP nop in front of it already blocks the SP SEQ on the
    # same tick, so the ring entry is enqueued only after its exp retired.
    observed_stores = []
    n_out = 0

    with tile.TileContext(nc) as tc:
        with (
            tc.tile_pool(name="slab_e", bufs=1) as pe,
            tc.tile_pool(name="slab_o", bufs=1) as po,
            tc.tile_pool(name="diff", bufs=D_BUFS) as pd,
            tc.tile_pool(name="out", bufs=O_BUFS) as pout,
            tc.tile_pool(name="out_sliver", bufs=2) as pos,
        ):
            # Issue all three per-tile slab loads upfront on the SP HWDGE
            # queue: FIFO order means loads t1/t2 fill the DMA gap between
            # load t0 completing and the first store becoming ready. Load
            # t0 is split after row 4 so the first sub (offset (0,0) reads
            # rows 0..4 only) starts one row-load earlier.
            slabs = []
            for t in range(TILES):
                se = pe.tile([128, SLAB], bf16, tag=f"se{t}")
                if t == 0:
                    nc.sync.dma_start(
                        out=se[:, 0:5 * PW],
                        in_=bass.AP(x_h, 0, [[2 * PW, 128], [1, 5 * PW]]))
                    nc.sync.dma_start(
                        out=se[:, 5 * PW:SLAB],
                        in_=bass.AP(x_h, 5 * PW,
                                    [[2 * PW, 128], [1, SLAB - 5 * PW]]))
                else:
                    nc.sync.dma_start(
                        out=se[:],
                        in_=bass.AP(x_h, t * IN_TILE,
                                    [[2 * PW, 128], [1, SLAB]]))
                slabs.append(se)

            for t in range(TILES):
                se = slabs[t]
                ve = se[:].rearrange("p (r c) -> p r c", c=PW)
                xi = ve[:, 2:4, 2:2 + W]

                # Tile 0 gates the first store (DMA idles until it issues),
                # so run even-dx offsets first there: their subs read only
                # the raw slab and the chain load->sub->mul->exp->store is
                # as short as possible. The +1-shifted odd-dx copy slots in
                # right after the first sub. Later tiles keep the copy
                # first so it is the tile's single DVE-side load observer
                # (all their subs already carry a d-slot-recycle ACT wait,
                # and walrus encodes only ONE sem wait per DVE instruction).
                if t == 0:
                    order = ([(k, o) for k, o in enumerate(OFFSETS)
                              if o[1] % 2 == 0] +
                             [(k, o) for k, o in enumerate(OFFSETS)
                              if o[1] % 2 == 1])
                    copy_pos = 5
                else:
                    order = list(enumerate(OFFSETS))
                    copy_pos = 0

                so = po.tile([128, SLAB], bf16, tag=f"so{t}")
                vo = so[:].rearrange("p (r c) -> p r c", c=PW)
                ci = None
                prev = None
                for j, (k, (dy, dx)) in enumerate(order):
                    if j == copy_pos:
                        ci = nc.vector.tensor_copy(so[:, 0:3094],
                                                   se[:, 1:3095])
                        if prev is not None:
                            tile.add_dep_helper(ci.ins, prev.ins, sync=False,
                                                reason="copy after 1st sub")
                    if dx % 2 == 0:
                        xj = ve[:, dy:dy + 2, dx:dx + W]
                    else:
                        xj = vo[:, dy:dy + 2, dx - 1:dx - 1 + W]
                    d = pd.tile([128, 2 * W], bf16, tag="d")
                    sliver = t == 0 and j < 2
                    # Sliver o-tiles come from a dedicated never-recycled
                    # pool: their two per-row stores sit on two different
                    # HWDGE lanes, and a recycling exp would inherit both
                    # lane waits (walrus-limited).
                    o = (pos if sliver else pout).tile(
                        [128, 1024], f32, tag="os" if sliver else "o")
                    if not sliver:
                        n_out += 1
                        if n_out > O_BUFS:
                            # o-slot recycle: the exp may not overwrite the
                            # slot before the store 10 offsets back drained
                            # it. Walrus fits only ONE sem wait on the exp
                            # (spent on its DVE RAW), so a 2-element ACT
                            # absorber takes the store's DMAHW wait first;
                            # the exp then elides it as already observed.
                            nc.scalar.copy(o[:, 0:2], o[:, 2:4])
                    dst_off = (t * 24 + k) * OUT_TILE
                    # The first two offsets run row-by-row: halving their
                    # sub/mul/exp chains gets the first stores issued
                    # ~1us sooner, right as the t1/t2 loads drain off the
                    # DMA engines.
                    rows = ((0, 1), (1, 2)) if sliver else ((0, 2),)
                    for r0, r1 in rows:
                        n = (r1 - r0) * W
                        si = nc.vector.tensor_sub(
                            d[:, r0 * W:r1 * W].rearrange(
                                "p (r c) -> p r c", c=W),
                            xj[:, r0:r1, :], xi[:, r0:r1, :])
                        if ci is not None:
                            # Ordering-only edge: keep subs after the
                            # odd-copy on DVE so its load-DMA wait is
                            # observed first.
                            tile.add_dep_helper(si.ins, ci.ins, sync=False,
                                                reason="sub after observer")
                        prev = si
                        nc.vector.tensor_mul(d[:, r0 * W:r1 * W],
                                             d[:, r0 * W:r1 * W],
                                             d[:, r0 * W:r1 * W])
                        ei = nc.scalar.activation(
                            o[:, r0 * W:r1 * W], d[:, r0 * W:r1 * W],
                            mybir.ActivationFunctionType.Exp, scale=-0.5)
                        dst = bass.AP(y_h, dst_off + r0 * W,
                                      [[1024, 128], [1, n]])
                        # With >8 DMAs in flight the HWDGE lane ring wait
                        # is unavoidable, and walrus can encode only ONE
                        # sem wait per DMA instruction. An SP nop absorbs
                        # the exp's ACT sem wait first, so the SP SEQ has
                        # observed that tick and the store itself keeps
                        # only its lane-FIFO wait.
                        ni = nc.sync.nop()
                        tile.add_dep_helper(ni.ins, ei.ins, sync=True,
                                            reason="observe exp on SP")
                        sti = nc.sync.dma_start(out=dst,
                                                in_=o[:, r0 * W:r1 * W])
                        tile.add_dep_helper(sti.ins, ni.ins, sync=False,
                                            reason="store after observer")
                        observed_stores.append((ni.ins, sti.ins))

    # Walrus encodes at most ONE sync wait per DMA instruction, but with
    # >8 HWDGE DMAs every store carries a lane-FIFO ring wait on top of
    # its exp data wait. The SP nop ahead of each store already waits on
    # the exp's ACT tick, so the SP SEQ cannot enqueue the ring entry
    # before its data is ready — drop the store's own ACT wait.
    for ni, sti in observed_stores:
        nw = {(str(w.ant_name), w.wait_value) for w in ni.sync_info.on_wait}
        keep, dropped = [], []
        for w in sti.sync_info.on_wait:
            if str(w.ant_name).startswith("Activation") and \
                    (str(w.ant_name), w.wait_value) in nw:
                dropped.append(w)
            else:
                keep.append(w)
        assert len(keep) <= 1, (
            f"store {sti.name} still has {len(keep)} waits: "
            f"{[(str(w.ant_name), w.wait_value) for w in keep]}")
        if dropped:
            sti.sync_info.on_wait = keep

    # An engine instruction waiting on its OWN engine's completion sem is
    # implied by in-order execution (the tick is necessarily in its past —
    # a future tick would deadlock): the exps inherit such WAW waits from
    # o-slot recycling, the subs/muls from d-slot rotation. Strip them so
    # each engine instruction keeps only real cross-engine waits. DMAs are
    # left alone — their waits execute asynchronously in the HWDGE ring,
    # where issuing-engine order implies nothing.
    own_sem = {"TensorTensor": "DVE", "TensorCopy": "DVE",
               "Activation": "Activation"}
    for blk in nc.m.functions[0].blocks:
        for ins in blk.instructions:
            pfx = own_sem.get(str(ins.opcode))
            if pfx is None or ins.sync_info is None:
                continue
            keep = [w for w in ins.sync_info.on_wait
                    if not str(w.ant_name).startswith(pfx)]
            if len(keep) != len(ins.sync_info.on_wait):
                ins.sync_info.on_wait = keep
    return nc


def _get_bass():
    global _CACHED
    if _CACHED is None:
        _CACHED = _build_bass()
    return _CACHED


def _shard_inputs(X: np.ndarray):
    """Full X [4,3,512,512] -> per-core flat padded half-image stacks (bf16)."""
    import ml_dtypes
    Xi = np.ascontiguousarray(X, dtype=np.float32).reshape(B * C, H, W)
    Xp = np.pad(Xi, ((0, 0), (PAD, PAD), (PAD, PAD))).astype(ml_dtypes.bfloat16)
    in_maps = []
    for c in range(N_CORES):
        arr = np.zeros([IN_LEN], dtype=ml_dtypes.bfloat16)
        for t in range(TILES):
            g = TILES * c + t
            m, r0 = g // 2, (g % 2) * HALF
            flat = Xp[m, r0:r0 + HALF + 2 * PAD, :].reshape(-1)
            arr[t * IN_TILE:(t + 1) * IN_TILE] = flat[:IN_TILE]
        in_maps.append({"x": arr})
    return in_maps


def _unshard_outputs(results):
    K = np.empty((B * C, 24, H, W), dtype=np.float32)
    for c in range(N_CORES):
        out = results[c]["y"].reshape(TILES, 24, HALF, W)
        for t in range(TILES):
            g = TILES * c + t
            m, r0 = g // 2, (g % 2) * HALF
            K[m, :, r0:r0 + HALF, :] = out[t]
    return K.reshape(B, C, 24, H, W)


def run(X: np.ndarray, trace: bool = False):
    nc = _get_bass()
    in_maps = _shard_inputs(X)
    res = run_bass_kernel_spmd(nc, in_maps, list(range(N_CORES)), trace=trace)
    return _unshard_outputs(res.results), res


def kernel(X: np.ndarray) -> np.ndarray:
    out, _ = run(X, trace=False)
    return out
